# revision 13
# baseline (speedup 1.0000x reference)
"""Trainium2 Bass kernel for the 4-layer LSTM (T=128, B=64, H=1024).

Strategy: 4-stage layer pipeline x 2-way batch data-parallel = 8 cores.
Rank r: stage j = r % 4 (layer j), batch half = r // 4 (B_LOC = 32).
The two batch halves run identical, fully independent pipelines.

Transport: AllGathers serialize on the collective firmware chain at
~35-55us per op regardless of payload, so chunks are shipped in 2-tick
rounds (one gather per 2 ticks, carrying 2 chunks = 4 steps). Gather
outputs rotate through 3 buffers (8-rank shared-output, 4-rank, 8-rank)
because a collective's rewrite of its output buffer is NOT ordered
against reader DMAs — the rotation gives consumers a 2-round safe read
window. Stage j+1 consumes stage j's chunk OFF=6 ticks after
production, so a gather has ~2 ticks of slack before its first
consumer (measured optimum: OFF=6 beats 7 and 8 — extra fill ticks
cost more than the residual gather-wait stalls they remove).

Compute per tick (G=2 steps), all in transposed space (zT = [4H, B_LOC],
no per-step transposes):
  - G sequential LSTM steps: h@U accumulates on top of the pre-computed
    x@W chunk in PSUM (start=False), then the sigmoid/tanh gate chain
    runs on Scalar/Vector.
  - The NEXT tick's batched x@W matmuls are interleaved between the
    per-step h@U blocks, so the PE (in-order) streams independent work
    during the gate chains. PSUM is double-buffered (4 banks per tick).
  - Each step's cT (bf16) goes to the round's DRAM bounce slot.

Output = cell state of layer 3 at t=T-1 (rank 3 holds batch 0:32,
rank 7 holds batch 32:64).
"""

import sys

for p in ("/opt/trn_rl_repo",):
    if p not in sys.path:
        sys.path.insert(0, p)

import numpy as np
import ml_dtypes

T, B, H, L = 128, 64, 1024, 4
FH = 4 * H
KT = H // 128           # 8 K-tiles
MT = FH // 128          # 32 M-tiles
B_LOC = B // 2          # batch per core (2-way data parallel)
G = 2                   # steps per chunk (one PSUM tick)
NCH = T // G            # chunks per layer
OFF = 3                 # tick offset between consecutive stages
C0 = 2                  # first active tick of stage 0
NTICKS = NCH + OFF * (L - 1) + C0   # stage j active [C0+OFF*j, C0+OFF*j+NCH-1]
N_CORES = 8
GB = G * B_LOC          # chunk free-dim (steps x local batch)
MMPB = 512 // GB        # mm blocks per 2KB PSUM bank

_CACHE = {}


def _build(nticks=NTICKS):
    import concourse.bacc as bacc
    import concourse.bass as bass
    import concourse.mybir as mybir
    import concourse.tile as tile

    bf16, f32, i32 = mybir.dt.bfloat16, mybir.dt.float32, mybir.dt.int32
    AF = mybir.ActivationFunctionType
    Alu = mybir.AluOpType

    nc = bacc.Bacc("TRN2", target_bir_lowering=False, debug=False,
                   num_devices=N_CORES)

    w_in = nc.dram_tensor("w_loc", [H, FH], bf16, kind="ExternalInput")
    u_in = nc.dram_tensor("u_loc", [H, FH], bf16, kind="ExternalInput")
    # partition-major: row p holds [chunk, k, g, b] (512B contiguous/chunk)
    src_static = nc.dram_tensor("src_static", [128, NCH * KT * GB], bf16,
                                kind="ExternalInput")
    rparam = nc.dram_tensor("rparam", [1, 2], i32, kind="ExternalInput")
    out_ext = nc.dram_tensor("out", [128, KT * B_LOC], f32,
                             kind="ExternalOutput")

    # Round-sized DRAM bounce buffers (a round = 2 ticks = 2 chunks), all
    # partition-major ([128, sub-chunk, k, n] rows) so every transport DMA
    # moves contiguous 512B-per-partition runs instead of 64B scraps.
    # c_out double-buffered by round parity; the gather target alternates
    # by gather-round parity between the two comms.
    CW = KT * GB          # one chunk's bytes-per-partition (in elements)
    c_out = [nc.dram_tensor(f"c_out{i}", [128, 2 * CW], bf16)
             for i in range(2)]
    # 3-deep gather rotation: a buffer is rewritten 3 rounds after it was
    # written, giving OFF=8 consumers a safe 2-round read window (Tile does
    # not order a collective's rewrite against reader DMAs).
    gbufs = [
        nc.dram_tensor("gath8a", [8, 128, 2 * CW], bf16,
                       addr_space="Shared"),
        nc.dram_tensor("gath4", [4, 128, 2 * CW], bf16),
        nc.dram_tensor("gath8b", [8, 128, 2 * CW], bf16,
                       addr_space="Shared"),
    ]

    with tile.TileContext(nc) as tc:
        with (
            tc.tile_pool(name="wp", bufs=1) as wp,
            tc.tile_pool(name="sp", bufs=1) as sp,
            tc.tile_pool(name="srcp", bufs=3) as srcp,
            tc.tile_pool(name="ewp", bufs=2) as ewp,
            tc.tile_pool(name="zp", bufs=2, space="PSUM") as zp,
        ):
            # ---- preamble -------------------------------------------------
            w_sb = wp.tile([128, KT * FH], bf16)   # W K-tile k at k*FH
            u_sb = wp.tile([128, KT * FH], bf16)
            for k in range(KT):
                nc.sync.dma_start(w_sb[:, k * FH:(k + 1) * FH],
                                  w_in[k * 128:(k + 1) * 128, :])
                nc.sync.dma_start(u_sb[:, k * FH:(k + 1) * FH],
                                  u_in[k * 128:(k + 1) * 128, :])

            rp_sb = sp.tile([1, 2], i32)
            nc.sync.dma_start(rp_sb[:], rparam[:])
            rv = nc.values_load(rp_sb[:1, 0:1].to_broadcast((1, 1)))
            rk = nc.values_load(rp_sb[:1, 1:2].to_broadcast((1, 1)))

            zsb = sp.tile([128, 2 * CW], bf16)
            nc.gpsimd.memset(zsb[:], 0.0)
            for i in range(2):
                nc.sync.dma_start(c_out[i][:, :], zsb[:])
            for gb, nslots in ((gbufs[0], 8), (gbufs[1], 4), (gbufs[2], 8)):
                for s in range(nslots):
                    nc.sync.dma_start(gb[s][:, :], zsb[:])

            # state (double-buffered by global step parity)
            cT = [sp.tile([128, KT * B_LOC], f32, name=f"cT{i}")
                  for i in range(2)]
            hT = [sp.tile([128, KT * B_LOC], bf16, name=f"hT{i}")
                  for i in range(2)]
            for i in range(2):
                nc.gpsimd.memset(cT[i][:], 0.0)
                nc.gpsimd.memset(hT[i][:], 0.0)

            # src chunk for consuming tick `tc_` (issued 2 ticks early):
            # stage 0 reads src_static chunk tc_-C0; stage j>0 reads the
            # chunk its predecessor produced at tick tc_-OFF from the
            # gather of round tp//2+1 (comm8 on even gather rounds).
            def issue_src(tc_):
                src_sb = srcp.tile([128, KT * GB], bf16, tag="src",
                                   name=f"src_{tc_}")
                kchunk = min(max(tc_ - C0, 0), NCH - 1)
                tp = max(tc_ - OFF, 0)
                sub = tp % 2
                m_g = tp // 2 + 1
                sel = m_g % 3
                use8 = sel != 1
                gt = gbufs[sel]
                with tc.If(rv == 0) as cmp:
                    nc.sync.dma_start(
                        src_sb[:],
                        src_static[:, kchunk * CW:(kchunk + 1) * CW])
                with cmp.Else():
                    ranks = (1, 2, 3, 5, 6, 7) if use8 else (1, 2, 3)
                    reg = rk if use8 else rv
                    for r in ranks:
                        with tc.If(reg == r):
                            nc.sync.dma_start(
                                src_sb[:],
                                gt[r - 1][:, sub * CW:(sub + 1) * CW])
                return src_sb

            # batched x@W for mm tiles [mmlo, mmhi) of a chunk. PSUM
            # start/stop are bank-granular: only the first matmul touching
            # a bank carries start=True (clears the bank's has_written).
            def issue_xw(psz_t, src_t, mmlo, mmhi):
                for mm in range(mmlo, mmhi):
                    for k in range(KT):
                        nc.tensor.matmul(
                            psz_t[:, mm * GB:(mm + 1) * GB],
                            w_sb[:, k * FH + mm * 128:k * FH + (mm + 1) * 128],
                            src_t[:, k * GB:(k + 1) * GB],
                            start=(mm % MMPB == 0 and k == 0), stop=False,
                            skip_group_check=True,
                        )

            gstep = 0  # global step counter for state parity

            srcs = {0: issue_src(0), 1: issue_src(1)}
            psz_cur = zp.tile([128, MT * GB], f32, tag="Z", name="psz_0")
            issue_xw(psz_cur, srcs[0], 0, MT)

            # ---- tick loop ------------------------------------------------
            for tau in range(nticks):
                if tau % 2 == 0:
                    m = tau // 2
                    sel = m % 3
                    comm8 = sel != 1
                    nc.gpsimd.collective_compute(
                        "AllGather", Alu.bypass,
                        replica_groups=([[0, 1, 2, 3, 4, 5, 6, 7]] if comm8
                                        else [[0, 1, 2, 3], [4, 5, 6, 7]]),
                        ins=[c_out[(m - 1) % 2].ap().opt()],
                        outs=[gbufs[sel].ap().opt()],
                    )

                # state reset at each stage's first active tick
                if tau >= C0 and (tau - C0) % OFF == 0 and (tau - C0) // OFF < L:
                    j = (tau - C0) // OFF
                    with tc.If(rv == j):
                        nc.gpsimd.memset(cT[gstep % 2][:], 0.0)
                        nc.gpsimd.memset(hT[gstep % 2][:], 0.0)

                if tau >= 1 and tau + 1 < nticks:
                    srcs[tau + 1] = issue_src(tau + 1)
                last = tau == nticks - 1
                if not last:
                    psz_next = zp.tile([128, MT * GB], f32, tag="Z",
                                       name=f"psz_{tau + 1}")

                cbf = ewp.tile([128, CW], bf16, tag="cbf",
                               name=f"cbf_{tau}")
                for s in range(G):
                    h_prev = hT[gstep % 2]
                    c_prev = cT[gstep % 2]
                    h_new = hT[(gstep + 1) % 2]
                    c_new = cT[(gstep + 1) % 2]
                    # h @ U accumulated on top of x@W (+start=False)
                    for mm in range(MT):
                        for k in range(KT):
                            nc.tensor.matmul(
                                psz_cur[:, mm * GB + s * B_LOC:
                                        mm * GB + (s + 1) * B_LOC],
                                u_sb[:, k * FH + mm * 128:
                                     k * FH + (mm + 1) * 128],
                                h_prev[:, k * B_LOC:(k + 1) * B_LOC],
                                start=False,
                                stop=(s == G - 1 and mm % MMPB == MMPB - 1
                                      and k == KT - 1),
                                skip_group_check=True,
                            )
                    # gates: mm 0-7 = i, 8-15 = f, 16-23 = g, 24-31 = o
                    # step-s columns: strided views [mm, s*B_LOC:(s+1)*B_LOC]
                    def zview(g0, g1, s=s):
                        return psz_cur[:].rearrange(
                            "p (mm n) -> p mm n", n=GB
                        )[:, g0 * 8:g1 * 8, s * B_LOC:(s + 1) * B_LOC]
                    sif = ewp.tile([128, 2 * KT * B_LOC], f32, tag="sif",
                                   name=f"sif_{tau}_{s}")
                    tg = ewp.tile([128, KT * B_LOC], f32, tag="tg",
                                  name=f"tg_{tau}_{s}")
                    so = ewp.tile([128, KT * B_LOC], f32, tag="so",
                                  name=f"so_{tau}_{s}")
                    nc.scalar.activation(
                        sif[:].rearrange("p (mm n) -> p mm n", n=B_LOC),
                        zview(0, 2), AF.Sigmoid)
                    nc.scalar.activation(
                        tg[:].rearrange("p (mm n) -> p mm n", n=B_LOC),
                        zview(2, 3), AF.Tanh)
                    nc.scalar.activation(
                        so[:].rearrange("p (mm n) -> p mm n", n=B_LOC),
                        zview(3, 4), AF.Sigmoid)
                    fc = ewp.tile([128, KT * B_LOC], f32, tag="fc",
                                  name=f"fc_{tau}_{s}")
                    ig = ewp.tile([128, KT * B_LOC], f32, tag="ig",
                                  name=f"ig_{tau}_{s}")
                    nc.vector.tensor_tensor(fc[:], sif[:, KT * B_LOC:],
                                            c_prev[:], Alu.mult)
                    nc.vector.tensor_tensor(ig[:], sif[:, 0:KT * B_LOC],
                                            tg[:], Alu.mult)
                    nc.vector.tensor_tensor(c_new[:], fc[:], ig[:], Alu.add)
                    th = ewp.tile([128, KT * B_LOC], f32, tag="th",
                                  name=f"th_{tau}_{s}")
                    nc.scalar.activation(th[:], c_new[:], AF.Tanh)
                    nc.vector.tensor_tensor(h_new[:], so[:], th[:], Alu.mult)
                    # next tick's x@W half: queued on the PE behind this
                    # step's h@U block, it streams while the gate chain
                    # above runs on Scalar/Vector.
                    if not last:
                        half = MT // G
                        issue_xw(psz_next, srcs[tau + 1],
                                 s * half, (s + 1) * half)
                    # cast c into the tick's transport tile (strided by k)
                    nc.vector.tensor_copy(
                        cbf[:].rearrange("p (k n) -> p k n", n=GB)
                        [:, :, s * B_LOC:(s + 1) * B_LOC],
                        c_new[:].rearrange("p (k n) -> p k n", n=B_LOC))
                    gstep += 1

                # one contiguous transport DMA per tick (both steps)
                nc.sync.dma_start(
                    c_out[(tau // 2) % 2]
                    [:, (tau % 2) * CW:(tau % 2 + 1) * CW],
                    cbf[:])

                if not last:
                    psz_cur = psz_next
                srcs.pop(tau, None)

            # final state out (ranks 3 and 7 hold the answer)
            nc.sync.dma_start(out_ext[:], cT[gstep % 2][:])
    nc.finalize()
    return nc


def _prep_in_maps(inputs, W, U, b):
    # partition-major src: [128, chunk, k, g, b] per batch half
    x5 = (inputs.astype(np.float32)
          .transpose(2, 0, 1)               # [H, T, B]
          .reshape(KT, 128, NCH, G, B)
          .transpose(1, 2, 0, 3, 4))        # [128, NCH, KT, G, B]
    halves = [
        np.ascontiguousarray(x5[:, :, :, :, :B_LOC]
                             .reshape(128, NCH * KT * GB))
        .astype(ml_dtypes.bfloat16),
        np.ascontiguousarray(x5[:, :, :, :, B_LOC:]
                             .reshape(128, NCH * KT * GB))
        .astype(ml_dtypes.bfloat16),
    ]
    zeros_src = np.zeros((128, NCH * KT * GB), dtype=ml_dtypes.bfloat16)
    Wb = W.astype(ml_dtypes.bfloat16)
    Ub = U.astype(ml_dtypes.bfloat16)
    in_maps = []
    for r in range(N_CORES):
        j = r % 4
        in_maps.append({
            "w_loc": np.ascontiguousarray(Wb[j]),
            "u_loc": np.ascontiguousarray(Ub[j]),
            "src_static": halves[r // 4] if j == 0 else zeros_src,
            "rparam": np.array([[j, r]], dtype=np.int32),
        })
    return in_maps


def kernel(inputs, W, U, b):
    assert not np.any(b), "nonzero bias not implemented"
    from concourse.bass_utils import run_bass_kernel_spmd

    if "nc" not in _CACHE:
        _CACHE["nc"] = _build()
    nc = _CACHE["nc"]
    in_maps = _prep_in_maps(inputs, W, U, b)
    res = run_bass_kernel_spmd(nc, in_maps, core_ids=list(range(N_CORES)))
    c = np.zeros((B, H), dtype=np.float32)
    for half, rank in ((0, 3), (1, 7)):
        ct = res.results[rank]["out"]  # [128, KT*B_LOC], k-tile k at k*B_LOC
        for k in range(KT):
            c[half * B_LOC:(half + 1) * B_LOC, k * 128:(k + 1) * 128] = \
                ct[:, k * B_LOC:(k + 1) * B_LOC].T
    return c



# revision 15
# speedup vs baseline: 1.0624x; 1.0624x over previous
"""Trainium2 Bass kernel for the 4-layer LSTM (T=128, B=64, H=1024).

Strategy: 4-stage layer pipeline x 2-way batch data-parallel = 8 cores.
Rank r: stage j = r % 4 (layer j), batch half = r // 4 (B_LOC = 32).
The two batch halves run identical, fully independent pipelines.

Transport: AllGathers serialize on the collective firmware chain at
~35-55us per op regardless of payload, so chunks are shipped in 2-tick
rounds (one gather per 2 ticks, carrying 2 chunks = 4 steps). Gather
outputs rotate through 3 buffers (8-rank shared-output, 4-rank, 8-rank)
because a collective's rewrite of its output buffer is NOT ordered
against reader DMAs — the rotation gives consumers a 2-round safe read
window. Stage j+1 consumes stage j's chunk OFF=6 ticks after
production, so a gather has ~2 ticks of slack before its first
consumer (measured optimum: OFF=6 beats 7 and 8 — extra fill ticks
cost more than the residual gather-wait stalls they remove).

Compute per tick (G=2 steps), all in transposed space (zT = [4H, B_LOC],
no per-step transposes):
  - G sequential LSTM steps: h@U accumulates on top of the pre-computed
    x@W chunk in PSUM (start=False), then the sigmoid/tanh gate chain
    runs on Scalar/Vector.
  - The NEXT tick's batched x@W matmuls are interleaved between the
    per-step h@U blocks, so the PE (in-order) streams independent work
    during the gate chains. PSUM is double-buffered (4 banks per tick).
  - Each step's cT (bf16) goes to the round's DRAM bounce slot.

Output = cell state of layer 3 at t=T-1 (rank 3 holds batch 0:32,
rank 7 holds batch 32:64).
"""

import sys

for p in ("/opt/trn_rl_repo",):
    if p not in sys.path:
        sys.path.insert(0, p)

import numpy as np
import ml_dtypes

T, B, H, L = 128, 64, 1024, 4
FH = 4 * H
KT = H // 128           # 8 K-tiles
MT = FH // 128          # 32 M-tiles
B_LOC = B // 2          # batch per core (2-way data parallel)
G = 2                   # steps per chunk (one PSUM tick)
NCH = T // G            # chunks per layer
OFF = 4                 # tick offset between consecutive stages
C0 = 1                  # first active tick of stage 0
NTICKS = NCH + OFF * (L - 1) + C0   # stage j active [C0+OFF*j, C0+OFF*j+NCH-1]
N_CORES = 8
GB = G * B_LOC          # chunk free-dim (steps x local batch)
MMPB = 512 // GB        # mm blocks per 2KB PSUM bank

_CACHE = {}


def _build(nticks=NTICKS):
    import concourse.bacc as bacc
    import concourse.bass as bass
    import concourse.mybir as mybir
    import concourse.tile as tile

    bf16, f32, i32 = mybir.dt.bfloat16, mybir.dt.float32, mybir.dt.int32
    AF = mybir.ActivationFunctionType
    Alu = mybir.AluOpType

    nc = bacc.Bacc("TRN2", target_bir_lowering=False, debug=False,
                   num_devices=N_CORES)

    w_in = nc.dram_tensor("w_loc", [H, FH], bf16, kind="ExternalInput")
    u_in = nc.dram_tensor("u_loc", [H, FH], bf16, kind="ExternalInput")
    # partition-major: row p holds [chunk, k, g, b] (512B contiguous/chunk)
    src_static = nc.dram_tensor("src_static", [128, NCH * KT * GB], bf16,
                                kind="ExternalInput")
    rparam = nc.dram_tensor("rparam", [1, 2], i32, kind="ExternalInput")
    out_ext = nc.dram_tensor("out", [128, KT * B_LOC], f32,
                             kind="ExternalOutput")

    # Round-sized DRAM bounce buffers (a round = 2 ticks = 2 chunks), all
    # partition-major ([128, sub-chunk, k, n] rows) so every transport DMA
    # moves contiguous 512B-per-partition runs instead of 64B scraps.
    # c_out double-buffered by round parity; the gather target alternates
    # by gather-round parity between the two comms.
    CW = KT * GB          # one chunk's bytes-per-partition (in elements)
    c_out = [nc.dram_tensor(f"c_out{i}", [128, 2 * CW], bf16)
             for i in range(2)]
    # 3-deep gather rotation: a buffer is rewritten 3 rounds after it was
    # written, giving OFF=8 consumers a safe 2-round read window (Tile does
    # not order a collective's rewrite against reader DMAs).
    gbufs = [
        nc.dram_tensor("gath8a", [8, 128, 2 * CW], bf16,
                       addr_space="Shared"),
        nc.dram_tensor("gath4", [4, 128, 2 * CW], bf16),
        nc.dram_tensor("gath8b", [8, 128, 2 * CW], bf16,
                       addr_space="Shared"),
    ]

    with tile.TileContext(nc) as tc:
        with (
            tc.tile_pool(name="wp", bufs=1) as wp,
            tc.tile_pool(name="sp", bufs=1) as sp,
            tc.tile_pool(name="srcp", bufs=3) as srcp,
            tc.tile_pool(name="ewp", bufs=2) as ewp,
            tc.tile_pool(name="zp", bufs=2, space="PSUM") as zp,
        ):
            # ---- preamble -------------------------------------------------
            w_sb = wp.tile([128, KT * FH], bf16)   # W K-tile k at k*FH
            u_sb = wp.tile([128, KT * FH], bf16)
            for k in range(KT):
                nc.sync.dma_start(w_sb[:, k * FH:(k + 1) * FH],
                                  w_in[k * 128:(k + 1) * 128, :])
                nc.sync.dma_start(u_sb[:, k * FH:(k + 1) * FH],
                                  u_in[k * 128:(k + 1) * 128, :])

            rp_sb = sp.tile([1, 2], i32)
            nc.sync.dma_start(rp_sb[:], rparam[:])
            rv = nc.values_load(rp_sb[:1, 0:1].to_broadcast((1, 1)))
            rk = nc.values_load(rp_sb[:1, 1:2].to_broadcast((1, 1)))

            zsb = sp.tile([128, 2 * CW], bf16)
            nc.gpsimd.memset(zsb[:], 0.0)
            for i in range(2):
                nc.sync.dma_start(c_out[i][:, :], zsb[:])
            for gb, nslots in ((gbufs[0], 8), (gbufs[1], 4), (gbufs[2], 8)):
                for s in range(nslots):
                    nc.sync.dma_start(gb[s][:, :], zsb[:])

            # state (double-buffered by global step parity)
            cT = [sp.tile([128, KT * B_LOC], f32, name=f"cT{i}")
                  for i in range(2)]
            hT = [sp.tile([128, KT * B_LOC], bf16, name=f"hT{i}")
                  for i in range(2)]
            for i in range(2):
                nc.gpsimd.memset(cT[i][:], 0.0)
                nc.gpsimd.memset(hT[i][:], 0.0)

            # src chunk for consuming tick `tc_` (issued 2 ticks early):
            # stage 0 reads src_static chunk tc_-C0; stage j>0 reads the
            # chunk its predecessor produced at tick tc_-OFF from the
            # gather of round tp//2+1 (comm8 on even gather rounds).
            def issue_src(tc_):
                src_sb = srcp.tile([128, KT * GB], bf16, tag="src",
                                   name=f"src_{tc_}")
                kchunk = min(max(tc_ - C0, 0), NCH - 1)
                tp = max(tc_ - OFF, 0)
                sub = tp % 2
                m_g = tp // 2 + 1
                sel = m_g % 3
                use8 = sel != 1
                gt = gbufs[sel]
                with tc.If(rv == 0) as cmp:
                    nc.sync.dma_start(
                        src_sb[:],
                        src_static[:, kchunk * CW:(kchunk + 1) * CW])
                with cmp.Else():
                    ranks = (1, 2, 3, 5, 6, 7) if use8 else (1, 2, 3)
                    reg = rk if use8 else rv
                    for r in ranks:
                        with tc.If(reg == r):
                            nc.sync.dma_start(
                                src_sb[:],
                                gt[r - 1][:, sub * CW:(sub + 1) * CW])
                return src_sb

            # batched x@W for mm tiles [mmlo, mmhi) of a chunk. PSUM
            # start/stop are bank-granular: only the first matmul touching
            # a bank carries start=True (clears the bank's has_written).
            def issue_xw(psz_t, src_t, mmlo, mmhi):
                for mm in range(mmlo, mmhi):
                    for k in range(KT):
                        nc.tensor.matmul(
                            psz_t[:, mm * GB:(mm + 1) * GB],
                            w_sb[:, k * FH + mm * 128:k * FH + (mm + 1) * 128],
                            src_t[:, k * GB:(k + 1) * GB],
                            start=(mm % MMPB == 0 and k == 0), stop=False,
                            skip_group_check=True,
                        )

            gstep = 0  # global step counter for state parity

            srcs = {0: issue_src(0), 1: issue_src(1)}
            psz_cur = zp.tile([128, MT * GB], f32, tag="Z", name="psz_0")
            issue_xw(psz_cur, srcs[0], 0, MT)

            # ---- tick loop ------------------------------------------------
            for tau in range(nticks):
                if tau % 2 == 0:
                    m = tau // 2
                    sel = m % 3
                    comm8 = sel != 1
                    nc.gpsimd.collective_compute(
                        "AllGather", Alu.bypass,
                        replica_groups=([[0, 1, 2, 3, 4, 5, 6, 7]] if comm8
                                        else [[0, 1, 2, 3], [4, 5, 6, 7]]),
                        ins=[c_out[(m - 1) % 2].ap().opt()],
                        outs=[gbufs[sel].ap().opt()],
                    )

                # state reset at each stage's first active tick
                if tau >= C0 and (tau - C0) % OFF == 0 and (tau - C0) // OFF < L:
                    j = (tau - C0) // OFF
                    with tc.If(rv == j):
                        nc.gpsimd.memset(cT[gstep % 2][:], 0.0)
                        nc.gpsimd.memset(hT[gstep % 2][:], 0.0)

                if tau + 2 < nticks:
                    srcs[tau + 2] = issue_src(tau + 2)
                last = tau == nticks - 1
                if not last:
                    psz_next = zp.tile([128, MT * GB], f32, tag="Z",
                                       name=f"psz_{tau + 1}")

                cbf = ewp.tile([128, CW], bf16, tag="cbf",
                               name=f"cbf_{tau}")
                for s in range(G):
                    h_prev = hT[gstep % 2]
                    c_prev = cT[gstep % 2]
                    h_new = hT[(gstep + 1) % 2]
                    c_new = cT[(gstep + 1) % 2]
                    # h @ U accumulated on top of x@W (+start=False)
                    for mm in range(MT):
                        for k in range(KT):
                            nc.tensor.matmul(
                                psz_cur[:, mm * GB + s * B_LOC:
                                        mm * GB + (s + 1) * B_LOC],
                                u_sb[:, k * FH + mm * 128:
                                     k * FH + (mm + 1) * 128],
                                h_prev[:, k * B_LOC:(k + 1) * B_LOC],
                                start=False,
                                stop=(s == G - 1 and mm % MMPB == MMPB - 1
                                      and k == KT - 1),
                                skip_group_check=True,
                            )
                    # gates: mm 0-7 = i, 8-15 = f, 16-23 = g, 24-31 = o
                    # step-s columns: strided views [mm, s*B_LOC:(s+1)*B_LOC]
                    def zview(g0, g1, s=s):
                        return psz_cur[:].rearrange(
                            "p (mm n) -> p mm n", n=GB
                        )[:, g0 * 8:g1 * 8, s * B_LOC:(s + 1) * B_LOC]
                    sif = ewp.tile([128, 2 * KT * B_LOC], f32, tag="sif",
                                   name=f"sif_{tau}_{s}")
                    tg = ewp.tile([128, KT * B_LOC], f32, tag="tg",
                                  name=f"tg_{tau}_{s}")
                    so = ewp.tile([128, KT * B_LOC], f32, tag="so",
                                  name=f"so_{tau}_{s}")
                    nc.scalar.activation(
                        sif[:].rearrange("p (mm n) -> p mm n", n=B_LOC),
                        zview(0, 2), AF.Sigmoid)
                    nc.scalar.activation(
                        tg[:].rearrange("p (mm n) -> p mm n", n=B_LOC),
                        zview(2, 3), AF.Tanh)
                    nc.scalar.activation(
                        so[:].rearrange("p (mm n) -> p mm n", n=B_LOC),
                        zview(3, 4), AF.Sigmoid)
                    fc = ewp.tile([128, KT * B_LOC], f32, tag="fc",
                                  name=f"fc_{tau}_{s}")
                    ig = ewp.tile([128, KT * B_LOC], f32, tag="ig",
                                  name=f"ig_{tau}_{s}")
                    nc.vector.tensor_tensor(fc[:], sif[:, KT * B_LOC:],
                                            c_prev[:], Alu.mult)
                    nc.vector.tensor_tensor(ig[:], sif[:, 0:KT * B_LOC],
                                            tg[:], Alu.mult)
                    nc.vector.tensor_tensor(c_new[:], fc[:], ig[:], Alu.add)
                    th = ewp.tile([128, KT * B_LOC], f32, tag="th",
                                  name=f"th_{tau}_{s}")
                    nc.scalar.activation(th[:], c_new[:], AF.Tanh)
                    nc.vector.tensor_tensor(h_new[:], so[:], th[:], Alu.mult)
                    # next tick's x@W half: queued on the PE behind this
                    # step's h@U block, it streams while the gate chain
                    # above runs on Scalar/Vector.
                    if not last:
                        half = MT // G
                        issue_xw(psz_next, srcs[tau + 1],
                                 s * half, (s + 1) * half)
                    # cast c into the tick's transport tile (strided by k)
                    nc.vector.tensor_copy(
                        cbf[:].rearrange("p (k n) -> p k n", n=GB)
                        [:, :, s * B_LOC:(s + 1) * B_LOC],
                        c_new[:].rearrange("p (k n) -> p k n", n=B_LOC))
                    gstep += 1

                # one contiguous transport DMA per tick (both steps)
                nc.sync.dma_start(
                    c_out[(tau // 2) % 2]
                    [:, (tau % 2) * CW:(tau % 2 + 1) * CW],
                    cbf[:])

                if not last:
                    psz_cur = psz_next
                srcs.pop(tau, None)

            # final state out (ranks 3 and 7 hold the answer)
            nc.sync.dma_start(out_ext[:], cT[gstep % 2][:])
    nc.finalize()
    return nc


def _prep_in_maps(inputs, W, U, b):
    # partition-major src: [128, chunk, k, g, b] per batch half
    x5 = (inputs.astype(np.float32)
          .transpose(2, 0, 1)               # [H, T, B]
          .reshape(KT, 128, NCH, G, B)
          .transpose(1, 2, 0, 3, 4))        # [128, NCH, KT, G, B]
    halves = [
        np.ascontiguousarray(x5[:, :, :, :, :B_LOC]
                             .reshape(128, NCH * KT * GB))
        .astype(ml_dtypes.bfloat16),
        np.ascontiguousarray(x5[:, :, :, :, B_LOC:]
                             .reshape(128, NCH * KT * GB))
        .astype(ml_dtypes.bfloat16),
    ]
    zeros_src = np.zeros((128, NCH * KT * GB), dtype=ml_dtypes.bfloat16)
    Wb = W.astype(ml_dtypes.bfloat16)
    Ub = U.astype(ml_dtypes.bfloat16)
    in_maps = []
    for r in range(N_CORES):
        j = r % 4
        in_maps.append({
            "w_loc": np.ascontiguousarray(Wb[j]),
            "u_loc": np.ascontiguousarray(Ub[j]),
            "src_static": halves[r // 4] if j == 0 else zeros_src,
            "rparam": np.array([[j, r]], dtype=np.int32),
        })
    return in_maps


def kernel(inputs, W, U, b):
    assert not np.any(b), "nonzero bias not implemented"
    from concourse.bass_utils import run_bass_kernel_spmd

    if "nc" not in _CACHE:
        _CACHE["nc"] = _build()
    nc = _CACHE["nc"]
    in_maps = _prep_in_maps(inputs, W, U, b)
    res = run_bass_kernel_spmd(nc, in_maps, core_ids=list(range(N_CORES)))
    c = np.zeros((B, H), dtype=np.float32)
    for half, rank in ((0, 3), (1, 7)):
        ct = res.results[rank]["out"]  # [128, KT*B_LOC], k-tile k at k*B_LOC
        for k in range(KT):
            c[half * B_LOC:(half + 1) * B_LOC, k * 128:(k + 1) * 128] = \
                ct[:, k * B_LOC:(k + 1) * B_LOC].T
    return c

